# revision 31
# baseline (speedup 1.0000x reference)
"""v5: fp16 shared expert + pure-fp8-DR routed experts on 8 trn2 cores.

Why (vs v3's compensated-fp8): on real TRN2 silicon a DoubleRow fp8
matmul nets only ~1.44x over bf16 (LDWEIGHTS pays +72%, MATMUL +13%; see
trainium-docs engines/01-tensor-engine.md), and only at free-dim >=256.
v3's "full" compensation (3 DR instructions per 2 k-tiles) is therefore
~1.5-1.7x SLOWER than a plain fp16 matmul on hardware, despite the cost
model charging it 0.75x.  Numerical emulation (emulate.py) shows:

  - pure (uncompensated) fp8 on the SHARED expert blows the 2e-2 error
    gate (~3e-2 alone: I_SH=4096 and every token): shared FFN runs fp16
    (error ~1e-3, real cost 1.0 cyc/col -- same silicon rate v3 paid for
    its compensated chains);
  - the ROUTED experts tolerate pure fp8 on gate+up (error attenuated by
    the top-2 routing weights p~0.3): those run fp8e4m3 DoubleRow at
    CG=512, the only regime where DR actually wins (~0.57x fp16);
  - expert down stays fp16.

Per-token scalars (router softmax p, shared sigmoid gate) are computed on
host: the host already computes routing indices and does the scatter-add
combine, so weighting rows there removes the device's exp/sigmoid/
reciprocal/broadcast machinery; the Activation engine runs a single table
set (silu+copy), eliminating v3's 84 activation-table reloads (~108us).

Sharding (unchanged): core c owns expert c (tokens host-gathered to
`cap` padded columns) plus a 1/8 tensor-parallel slice of the shared
expert over all tokens.  The shared-expert partial [T, H] is written
dense in fp16 (range-ordered, 8 ranges) and ReduceScattered per range,
overlapping the expert phase.  Host combine: out = sg * shared +
scatter-add(p_e * expert rows), all fp32.

Schedule notes (cost model: 97.6% PE occupancy, single 0.9us gap):
  - expert token counts are near-uniform (~2048 +/- 60), so `cap` is
    128-granular with a partial tail block instead of ceil-to-512
    padding (saves ~0.4 blocks/core of PE time);
  - startup DMA is bandwidth-ordered: first gate-weight half + first x
    block load before everything else, so the first matmul chain starts
    ~20us earlier; down-proj outputs stream out per 128-token slice so
    the ReduceScatter of each range launches as early as possible.
"""

import numpy as np
import ml_dtypes
from contextlib import ExitStack

import concourse.bass as bass
import concourse.bacc as bacc
import concourse.tile as tile
import concourse.mybir as mybir
from concourse import bass_utils
from concourse.bass_interp import get_hw_module

B, S, H = 2, 4096, 2048
E, TOP_K = 8, 2
I_EXP, I_SH = 1024, 4096
T = B * S
NCORES = 8
I_SLICE = I_SH // NCORES       # 512

P = 128
KT = H // P                    # 16 k-tiles over H
ITS = I_SLICE // P             # 4 i-tiles (shared slice)
ITE = I_EXP // P               # 8 i-tiles (expert)
TB = 512                       # token block
CG = 512                       # matmul moving free size
NBLK = T // TB                 # 16
NRANGE = 8
TRANGE = T // NRANGE           # 1024
SW = 128.0                     # fp8 weight scale (power of 2)

EU_P8 = True                   # expert up-proj in pure fp8 (else fp16)

F32 = mybir.dt.float32
F16 = mybir.dt.float16
F8 = mybir.dt.float8e4
F8ML = ml_dtypes.float8_e4m3
DR = mybir.MatmulPerfMode.DoubleRow
ALU = mybir.AluOpType
ACTF = mybir.ActivationFunctionType

RS_EACH_NS = 7_300 + int(TRANGE * H * 2 / 122e9 * 1e9)   # per-range fp16 RS


def dr_chain(nc, pt, wt, isl, xt, nkt, tb=TB):
    """PSUM chain of pure-fp8 DoubleRow matmuls: wt [P, nkt, I], xt [P, nkt, >=tb]."""
    for k in range(0, nkt, 2):
        nc.tensor.matmul(pt, wt[:, k : k + 2, isl * P : (isl + 1) * P],
                         xt[:, k : k + 2, 0:tb], start=(k == 0),
                         stop=(k == nkt - 2), perf_mode=DR)


def f16_chain(nc, pt, wt, isl, xt, nkt, tb=TB):
    """PSUM chain of fp16 matmuls: wt [P, nkt, I], xt [P, nkt, >=tb]."""
    for k in range(nkt):
        nc.tensor.matmul(pt, wt[:, k, isl * P : (isl + 1) * P],
                         xt[:, k, 0:tb], start=(k == 0), stop=(k == nkt - 1))


def down_chain(nc, pt, hpk, ts, w3, hsl, nit):
    """fp16 down-proj chain: stationary h tile [P, 128], moving W3T [P, CG]."""
    for i in range(nit):
        nc.tensor.matmul(pt, hpk[:, i, ts * P : (ts + 1) * P],
                         w3[:, i, hsl], start=(i == 0), stop=(i == nit - 1))


def _ffn_block(nc, ps, sbT, xb, xb2, w1, w2, w3, hpk, yb, nit, gu_chain,
               gu2_chain, sc_g, sc_y, tb=TB):
    """One token block of FFN (tb tokens, 128-multiple).  gu_chain/gu2_chain
    emit the gate/up GEMMs (fp8-DR or fp16); sc_g descales the fp8 gate for
    silu; sc_y descales the output (h carries the up-proj's fp8 scale)."""
    for isl in range(nit):
        psG = ps.tile([P, CG], F32, tag="psG")
        gu_chain(nc, psG[0:P, 0:tb], w1, isl, xb, KT, tb=tb)
        psU = ps.tile([P, CG], F32, tag="psU")
        gu2_chain(nc, psU[0:P, 0:tb], w2, isl, xb2, KT, tb=tb)
        sg = sbT.tile([P, CG], F32, tag="sg")
        nc.scalar.activation(sg[0:P, 0:tb], psG[0:P, 0:tb], ACTF.Silu,
                             scale=sc_g)
        nc.vector.tensor_tensor(hpk[:, isl, 0:tb], sg[0:P, 0:tb],
                                psU[0:P, 0:tb], ALU.mult)
    for ts in range(tb // P):
        for hh in range(H // CG):
            hsl = slice(hh * CG, (hh + 1) * CG)
            psY = ps.tile([P, CG], F32, tag="psY")
            down_chain(nc, psY[:], hpk, ts, w3, hsl, nit)
            nc.scalar.activation(yb[:, ts, hsl], psY[:], ACTF.Copy, scale=sc_y)
        yield ts


def build_kernel(cap, num_devices=NCORES, with_rs=True, do_expert=True,
                 do_shared=True):
    nbe = (cap + TB - 1) // TB          # last block may be partial (128-mult)

    nc = bacc.Bacc(
        "TRN2", target_bir_lowering=False, debug=False, enable_asserts=False,
        num_devices=num_devices, num_swdge_queues=4,
    )
    xT16 = nc.dram_tensor("xT16", [KT, P, T], F16, kind="ExternalInput").ap()
    xe8 = nc.dram_tensor("xe8", [KT, P, cap], F8, kind="ExternalInput").ap()
    xe16 = (None if EU_P8 else
            nc.dram_tensor("xe16", [KT, P, cap], F16, kind="ExternalInput").ap())
    # weights are host-pre-swizzled to partition-major so loads are contiguous
    ws1 = nc.dram_tensor("ws1", [P, KT, I_SLICE], F16, kind="ExternalInput").ap()
    ws2 = nc.dram_tensor("ws2", [P, KT, I_SLICE], F16, kind="ExternalInput").ap()
    ws3 = nc.dram_tensor("ws3", [P, ITS, H], F16, kind="ExternalInput").ap()
    we1 = nc.dram_tensor("we1", [P, KT, I_EXP], F8, kind="ExternalInput").ap()
    we2 = nc.dram_tensor(
        "we2", [P, KT, I_EXP], F8 if EU_P8 else F16, kind="ExternalInput").ap()
    we3 = nc.dram_tensor("we3", [P, ITE, H], F16, kind="ExternalInput").ap()
    out_shard = nc.dram_tensor(
        "out_shard", [NRANGE, TRANGE // NCORES, H], F16, kind="ExternalOutput"
    ).ap()
    exp_out = nc.dram_tensor("exp_out", [cap, H], F16, kind="ExternalOutput").ap()

    with tile.TileContext(nc) as tc, ExitStack() as ctx:
        dram = ctx.enter_context(tc.tile_pool(name="dram", bufs=1, space="DRAM"))
        partials = [
            dram.tile([TRANGE, H], F16, tag=f"partial{r}", name=f"partial{r}")
            for r in range(NRANGE)
        ]
        dramR = ctx.enter_context(tc.tile_pool(name="dramR", bufs=1, space="DRAM"))

        # weights resident in SBUF for the whole kernel
        # Startup is DMA-bandwidth-bound: issue loads in first-use order
        # (ws1 halves + first x block first; ws2/ws3 follow inside block 0).
        cst = ctx.enter_context(tc.tile_pool(name="cst", bufs=1))
        ws1sb = cst.tile([P, KT, I_SLICE], F16, tag="ws1sb")
        nc.sync.dma_start(ws1sb[:, 0 : KT // 2, :], ws1[:, 0 : KT // 2, :])
        ws2sb = cst.tile([P, KT, I_SLICE], F16, tag="ws2sb")
        ws3sb = cst.tile([P, ITS, H], F16, tag="ws3sb")

        def load_shared_tail():
            nc.sync.dma_start(ws2sb[:, 0 : KT // 2, :], ws2[:, 0 : KT // 2, :])
            nc.sync.dma_start(ws1sb[:, KT // 2 :, :], ws1[:, KT // 2 :, :])
            nc.sync.dma_start(ws2sb[:, KT // 2 :, :], ws2[:, KT // 2 :, :])
            nc.sync.dma_start(ws3sb[:], ws3[:])
        cstE = ctx.enter_context(tc.tile_pool(name="cstE", bufs=1))
        sbXE = ctx.enter_context(tc.tile_pool(name="sbXE", bufs=2))

        rs_done = [False] * NRANGE

        def issue_rs(r):
            if rs_done[r] or not do_shared:
                return
            rs_done[r] = True
            if with_rs:
                rs_out = dramR.tile(
                    [TRANGE // NCORES, H], F16, tag=f"rsout{r}", name=f"rsout{r}"
                )
                nc.gpsimd.collective_compute(
                    "ReduceScatter",
                    ALU.add,
                    replica_groups=[list(range(num_devices))],
                    ins=[partials[r][:, :].opt()],
                    outs=[rs_out.opt()],
                )
                nc.sync.dma_start(out_shard[r], rs_out[:])
            else:
                nc.sync.dma_start(
                    out_shard[r], partials[r][0 : TRANGE // NCORES, :]
                )

        def load_expert_weights():
            we1sb = cstE.tile([P, KT, I_EXP], F8, tag="we1sb")
            nc.sync.dma_start(we1sb[:], we1[:])
            we2sb = cstE.tile([P, KT, I_EXP], F8 if EU_P8 else F16, tag="we2sb")
            nc.sync.dma_start(we2sb[:], we2[:])
            we3sb = cstE.tile([P, ITE, H], F16, tag="we3sb")
            nc.sync.dma_start(we3sb[:], we3[:])
            return we1sb, we2sb, we3sb

        ew = []
        xpre_l = []
        if do_shared:
            with (
                tc.tile_pool(name="sbXS", bufs=2) as sbX,
                tc.tile_pool(name="sbHS", bufs=2) as sbH,
                tc.tile_pool(name="sbTS", bufs=3) as sbT,
                tc.tile_pool(name="sbYS", bufs=2) as sbY,
                tc.tile_pool(name="psS", bufs=2, space="PSUM") as ps,
            ):
                for b in range(NBLK):
                    bsl = slice(b * TB, (b + 1) * TB)
                    rng_i = (b * TB) // TRANGE
                    xb = sbX.tile([P, KT, TB], F16, tag="xbs")
                    if b == 0:
                        nc.sync.dma_start(
                            xb[:, 0 : KT // 2, :],
                            xT16[0 : KT // 2, :, bsl].rearrange("k p t -> p k t"),
                        )
                        nc.sync.dma_start(
                            xb[:, KT // 2 :, :],
                            xT16[KT // 2 :, :, bsl].rearrange("k p t -> p k t"),
                        )
                        load_shared_tail()
                    else:
                        nc.sync.dma_start(
                            xb[:], xT16[:, :, bsl].rearrange("k p t -> p k t")
                        )
                    if b == 1 and do_expert:
                        ew.append(load_expert_weights())
                    if b == NBLK - 2 and do_expert:
                        # prefetch the first expert x block so the expert
                        # phase's first chain starts without a DMA wait
                        tb0 = min(TB, cap)
                        xpre = sbXE.tile([P, KT, TB], F8, tag="xbe")
                        nc.sync.dma_start(
                            xpre[:, :, 0:tb0],
                            xe8[:, :, 0:tb0].rearrange("k p t -> p k t"),
                        )
                        xpre_l.append(xpre)
                    hpk = sbH.tile([P, ITS, TB], F16, tag="hspk")
                    yb = sbY.tile([P, TB // P, H], F16, tag="ybs")
                    row0 = (b * TB) % TRANGE
                    dst = partials[rng_i][row0 : row0 + TB, :].rearrange(
                        "(a p) h -> a p h", p=P
                    )
                    for ts in _ffn_block(nc, ps, sbT, xb, xb, ws1sb, ws2sb,
                                         ws3sb, hpk, yb, ITS, f16_chain,
                                         f16_chain, 1.0, 1.0):
                        nc.sync.dma_start(dst[ts], yb[:, ts, :])
                    if row0 + TB == TRANGE:
                        issue_rs(rng_i)
        for r in range(NRANGE):
            issue_rs(r)

        if do_expert:
            if not ew:
                ew.append(load_expert_weights())
            we1sb, we2sb, we3sb = ew[0]
            with (
                tc.tile_pool(name="sbHE", bufs=2) as sbH,
                tc.tile_pool(name="sbTE", bufs=3) as sbT,
                tc.tile_pool(name="sbYE", bufs=2) as sbY,
                tc.tile_pool(name="psE", bufs=2, space="PSUM") as ps,
            ):
                for eb in range(nbe):
                    tb = min(TB, cap - eb * TB)
                    bsl = slice(eb * TB, eb * TB + tb)
                    if eb == 0 and xpre_l:
                        xb = xpre_l[0]
                    else:
                        xb = sbXE.tile([P, KT, TB], F8, tag="xbe")
                        nc.sync.dma_start(
                            xb[:, :, 0:tb],
                            xe8[:, :, bsl].rearrange("k p t -> p k t"),
                        )
                    if EU_P8:
                        xb2 = xb
                        up_chain = dr_chain
                    else:
                        xb2 = sbXE.tile([P, KT, TB], F16, tag="xbe16")
                        nc.sync.dma_start(
                            xb2[:, :, 0:tb],
                            xe16[:, :, bsl].rearrange("k p t -> p k t"),
                        )
                        up_chain = f16_chain
                    hpk = sbH.tile([P, ITE, TB], F16, tag="hepk")
                    yb = sbY.tile([P, TB // P, H], F16, tag="ybe")
                    dst = exp_out[bsl, :].rearrange("(a p) h -> a p h", p=P)
                    for ts in _ffn_block(nc, ps, sbT, xb, xb2, we1sb, we2sb,
                                         we3sb, hpk, yb, ITE, dr_chain,
                                         up_chain, 1.0 / SW,
                                         1.0 / SW if EU_P8 else 1.0, tb=tb):
                        nc.sync.dma_start(dst[ts], yb[:, ts, :])

    nc.compile()
    return nc


# ---------------------------------------------------------------------------
# host side
# ---------------------------------------------------------------------------

def route_host(inputs):
    """Routing + per-token scalars in fp64/fp32 on host."""
    x = np.asarray(inputs["hidden_states"], np.float64).reshape(T, H)
    gw = np.asarray(inputs["gate_w"], np.float64)
    logits = x @ gw.T
    p = np.exp(logits - logits.max(-1, keepdims=True))
    p /= p.sum(-1, keepdims=True)
    order = np.argsort(-p, axis=-1, kind="stable")
    top2 = order[:, :TOP_K]
    toks_per_core, pw_per_core = [], []
    for e in range(NCORES):
        toks = np.where((top2 == e).any(-1))[0]
        toks_per_core.append(toks)
        pw_per_core.append(p[toks, e].astype(np.float32))
    cap = max(len(t) for t in toks_per_core)
    cap = int(np.ceil(cap / P) * P)     # 128-granular; kernel has a tail block
    segw = np.asarray(inputs["shared_expert_gate_w"], np.float64)
    sg = 1.0 / (1.0 + np.exp(-(x @ segw.T)))       # [T, 1]
    return toks_per_core, pw_per_core, cap, sg.astype(np.float32)


def pack_16(aT):
    """[K, N] fp32 -> [K//P, P, N] fp16 (k-tile-major, for x)."""
    a = np.asarray(aT, np.float32)
    return np.ascontiguousarray(
        a.astype(np.float16).reshape(a.shape[0] // P, P, a.shape[1]))


def pack_x8(xT):
    """[K, N] fp32 -> [K//P, P, N] fp8 (k-tile-major, for x)."""
    a = np.asarray(xT, np.float32)
    return np.ascontiguousarray(
        a.astype(F8ML).reshape(a.shape[0] // P, P, a.shape[1]))


def _pm(a):
    """k-tile-major [KT, P, M] -> partition-major [P, KT, M] (SBUF layout)."""
    return np.ascontiguousarray(np.transpose(a, (1, 0, 2)))


def pack_w8(wT, s=SW):
    """[K, M] fp32 -> [P, K//P, M] fp8 (x scale s), partition-major."""
    w = np.asarray(wT, np.float32) * s
    return _pm(w.astype(F8ML).reshape(w.shape[0] // P, P, w.shape[1]))


def pack_w16(wT):
    """[K, M] fp32 -> [P, K//P, M] fp16, partition-major."""
    w = np.asarray(wT, np.float32)
    return _pm(w.astype(np.float16).reshape(w.shape[0] // P, P, w.shape[1]))


def pack_w16T(w):
    """[M, K] fp32 -> W.T packed [P, K//P, M] fp16 (down-proj moving)."""
    wT = np.ascontiguousarray(np.asarray(w, np.float32).T)
    return _pm(wT.astype(np.float16).reshape(wT.shape[0] // P, P, wT.shape[1]))


def make_in_maps(inputs):
    x = np.ascontiguousarray(
        np.asarray(inputs["hidden_states"], np.float32).reshape(T, H))
    toks_per_core, pw_per_core, cap, sg = route_host(inputs)

    xT = x.T                                             # [H, T]
    xT16 = pack_16(xT)                                   # [KT, P, T]
    xT8 = pack_x8(xT)

    egw = np.asarray(inputs["expert_gate_w"], np.float32)
    euw = np.asarray(inputs["expert_up_w"], np.float32)
    edw = np.asarray(inputs["expert_down_w"], np.float32)
    sgw = np.asarray(inputs["shared_gate_w"], np.float32)
    suw = np.asarray(inputs["shared_up_w"], np.float32)
    sdw = np.asarray(inputs["shared_down_w"], np.float32)

    in_maps = []
    for c in range(NCORES):
        ssl = slice(c * I_SLICE, (c + 1) * I_SLICE)
        toks = toks_per_core[c]
        gcols = np.zeros(cap, np.int64)
        gcols[: len(toks)] = toks
        im = {
            "xT16": xT16,
            "xe8": np.ascontiguousarray(xT8[:, :, gcols]),
            "ws1": pack_w16(sgw[ssl].T),
            "ws2": pack_w16(suw[ssl].T),
            "ws3": pack_w16T(sdw[:, ssl]),
            "we1": pack_w8(egw[c].T),
            "we2": (pack_w8(euw[c].T) if EU_P8 else pack_w16(euw[c].T)),
            "we3": pack_w16T(edw[c]),
        }
        if not EU_P8:
            im["xe16"] = np.ascontiguousarray(xT16[:, :, gcols])
        in_maps.append(im)
    return in_maps, cap, toks_per_core, pw_per_core, sg


def assemble_output(results, toks_per_core, pw_per_core, sg):
    out = np.zeros((T, H), np.float32)
    rows = TRANGE // NCORES
    for c in range(NCORES):
        sh = results[c]["out_shard"]                     # [NRANGE, rows, H] f16
        for r in range(NRANGE):
            base = r * TRANGE + c * rows
            out[base : base + rows] = sh[r].astype(np.float32)
    out *= sg                                            # shared sigmoid gate
    for c in range(NCORES):
        toks = toks_per_core[c]
        ey = results[c]["exp_out"][: len(toks)].astype(np.float32)
        out[toks] += pw_per_core[c][:, None] * ey
    return out.reshape(B, S, H)


_nc_cache = {}


def kernel(**inputs) -> np.ndarray:
    in_maps, cap, toks_per_core, pw_per_core, sg = make_in_maps(inputs)
    if cap not in _nc_cache:
        nc = build_kernel(cap)
        nc.m = get_hw_module(nc.m)
        _nc_cache[cap] = nc
    nc = _nc_cache[cap]
    res = bass_utils.run_bass_kernel_spmd(
        nc, in_maps, core_ids=list(range(NCORES))
    )
    return assemble_output(res.results, toks_per_core, pw_per_core, sg)
